# revision 14
# baseline (speedup 1.0000x reference)
"""Trainium2 Bass kernel for shifted sparse attention (nn_Attention_74672301408506).

Math (reference):
    q = x @ W.T ; k = x_key @ W.T ; att = softmax(q k^T)
    out[i, v] = sum_s w_s * sum_j att[i-2s, j] * x_value[j+2s, v]

Device algorithm (8 cores, query rows sharded, 8 halo rows recomputed):
    per core, with local query window rows [r0-8, r0+512):
      qT[h, i], kT[h, j]   (float32r matmuls)
      S^T[j, i] = kT^T q   (scores transposed: keys on partitions)
      E = exp(S - 110)     (bf16; softmax shift-invariant, 110 avoids overflow)
      Ru[i, 0:320|320] = E^T @ [V_0|V_1|V_2|V_3|ones]   (ones col = softmax denom)
      R = Ru[:, :320] * mask / Ru[:, 320]
      out[p, v] = sum_s w_s R[p + 8 - 2s, 80s + v]  (banded-matrix matmuls)
"""

import os
import sys
import types

import numpy as np
import ml_dtypes

T = 4096
Q = 256
H = 256
NV = 80
NS = 4
STEP = 2
NCORES = 8
M = T // NCORES            # 512 rows per core
HALO = 8                   # = (NS-1)*STEP + slack, multiple of 2
MH = M + HALO              # 520 i-window per core
CSUB = 110.0               # global score shift before exp
P = 128
NJ = T // P                # 32 key tiles
NF = Q // P                # 2 feature tiles
NH = H // P                # 2 hidden tiles
NMAIN = M // P             # 4 main i-chunks of 128


def _install_axon_ntff_hook():
    """bass_utils' trace path imports antenv.axon_hooks, which the agent image
    lacks; shim it and register the ctypes-based NTFF profiler hook."""
    if "antenv.axon_hooks" in sys.modules:
        return
    try:
        import antenv
    except ImportError:
        return
    mod = types.ModuleType("antenv.axon_hooks")
    mod._hook = None
    mod.set_axon_ntff_profile_hook = lambda h: setattr(mod, "_hook", h)
    mod.get_axon_ntff_profile_hook = lambda: mod._hook
    sys.modules["antenv.axon_hooks"] = mod
    antenv.axon_hooks = mod
    try:
        from trn_agent_boot import trn_boot

        so_path = "/opt/axon/libaxon_pjrt.so"
        if os.path.exists(so_path):
            mod.set_axon_ntff_profile_hook(trn_boot._ntff_profile_via_ctypes(so_path))
    except Exception:
        pass


_NC_CACHE = {}
LAST_RESULT = None


def _build_nc():
    import concourse.mybir as mybir
    import concourse.tile as tile
    from concourse import bacc

    f32 = mybir.dt.float32
    f32r = mybir.dt.float32r
    bf16 = mybir.dt.bfloat16
    Exp = mybir.ActivationFunctionType.Exp

    nc = bacc.Bacc(None, target_bir_lowering=False)

    xT_d = nc.dram_tensor("xT", [Q, MH], f32r, kind="ExternalInput")
    xkT_d = nc.dram_tensor("xkT", [Q, T], f32r, kind="ExternalInput")
    wT_d = nc.dram_tensor("wT", [Q, H], f32r, kind="ExternalInput")
    vc_d = nc.dram_tensor("vcomb", [T, NS * NV + 1], bf16, kind="ExternalInput")
    sh1_d = nc.dram_tensor("shmat1", [P, NS * P], bf16, kind="ExternalInput")
    sh2_d = nc.dram_tensor("shmat2", [HALO, NS * P], bf16, kind="ExternalInput")
    rmask_d = nc.dram_tensor("rmask", [P, 1], f32, kind="ExternalInput")
    rminv_d = nc.dram_tensor("rminv", [P, 1], f32, kind="ExternalInput")
    out_d = nc.dram_tensor("out", [M, NV], f32, kind="ExternalOutput")

    with tile.TileContext(nc) as tc:
        with (
            tc.tile_pool(name="consts", bufs=1) as consts,
            tc.tile_pool(name="io", bufs=3) as io,
            tc.tile_pool(name="store", bufs=1) as store,
            tc.tile_pool(name="small", bufs=6) as small,
            tc.tile_pool(name="psA", bufs=2, space="PSUM") as psA,
            tc.tile_pool(name="psB", bufs=1, space="PSUM") as psB,
            tc.tile_pool(name="psR", bufs=4, space="PSUM") as psR,
            tc.tile_pool(name="psR8", bufs=1, space="PSUM") as psR8,
        ):
            # ---- constants / small inputs ----
            sh1 = consts.tile([P, NS * P], bf16, name="sh1")
            nc.sync.dma_start(out=sh1, in_=sh1_d[:, :])
            sh2 = consts.tile([HALO, NS * P], bf16, name="sh2")
            nc.sync.dma_start(out=sh2, in_=sh2_d[:, :])
            rmask = consts.tile([P, 1], f32, name="rmask")
            nc.sync.dma_start(out=rmask, in_=rmask_d[:, :])
            rminv = consts.tile([P, 1], f32, name="rminv")
            nc.sync.dma_start(out=rminv, in_=rminv_d[:, :])
            bias_t = consts.tile([P, 1], f32, name="bias_t")
            nc.vector.memset(bias_t, -CSUB)

            wt = []
            for f in range(NF):
                t = consts.tile([P, H], f32r, name=f"wt{f}", tag=f"wt{f}")
                nc.sync.dma_start(out=t, in_=wT_d[P * f : P * (f + 1), :])
                wt.append(t)
            xt = []
            for f in range(NF):
                t = consts.tile([P, MH], f32r, name=f"xt{f}", tag=f"xt{f}")
                nc.sync.dma_start(out=t, in_=xT_d[P * f : P * (f + 1), :])
                xt.append(t)

            # value tiles (bf16, includes ones column)
            vc = []
            for j in range(NJ):
                t = store.tile([P, NS * NV + 1], bf16, name=f"vc{j}", tag="vc", bufs=NJ)
                nc.sync.dma_start(out=t, in_=vc_d[P * j : P * (j + 1), :])
                vc.append(t)

            # ---- phase A: qT, kT ----
            qt = []
            for h in range(NH):
                ps = psA.tile([P, M], f32, name=f"qps{h}", tag="ps")
                for f in range(NF):
                    nc.tensor.matmul(
                        ps,
                        wt[f][:, P * h : P * (h + 1)],
                        xt[f][:, 0:M],
                        start=(f == 0),
                        stop=(f == NF - 1),
                    )
                ps8 = psB.tile([P, HALO], f32, name=f"qps8{h}", tag="ps8")
                for f in range(NF):
                    nc.tensor.matmul(
                        ps8,
                        wt[f][:, P * h : P * (h + 1)],
                        xt[f][:, M:MH],
                        start=(f == 0),
                        stop=(f == NF - 1),
                    )
                t = store.tile([P, MH], f32r, name=f"qt{h}", tag=f"qt{h}")
                if h == 0:
                    nc.vector.tensor_copy(t[:, 0:M], ps)
                else:
                    nc.scalar.copy(t[:, 0:M], ps)
                nc.vector.tensor_copy(t[:, M:MH], ps8)
                qt.append(t)

            kt = []
            for h in range(NH):
                kt.append(store.tile([P, T], f32r, name=f"kt{h}", tag=f"kt{h}"))
            NKC = 8  # 512-wide key chunks
            for jc in range(NKC):
                xkc = []
                for f in range(NF):
                    t = io.tile([P, 512], f32r, name=f"xkc{jc}_{f}", tag=f"xk{f}")
                    nc.sync.dma_start(
                        out=t, in_=xkT_d[P * f : P * (f + 1), 512 * jc : 512 * (jc + 1)]
                    )
                    xkc.append(t)
                for h in range(NH):
                    ps = psA.tile([P, 512], f32, name=f"kps{jc}_{h}", tag="ps")
                    for f in range(NF):
                        nc.tensor.matmul(
                            ps,
                            wt[f][:, P * h : P * (h + 1)],
                            xkc[f],
                            start=(f == 0),
                            stop=(f == NF - 1),
                        )
                    dst = kt[h][:, 512 * jc : 512 * (jc + 1)]
                    if (jc + h) % 2 == 0:
                        nc.vector.tensor_copy(dst, ps)
                    else:
                        nc.scalar.copy(dst, ps)

            # ---- phases B+C interleaved: scores^T -> exp -> Ru accumulation ----
            elist = []
            ru = []
            for c in range(NMAIN):
                ru.append(psR.tile([P, NS * NV + 1], f32, name=f"ru{c}", tag="ru"))
            ru8 = psR8.tile([HALO, NS * NV + 1], f32, name="ru8", tag="ru8")
            e8 = store.tile([P, 8 * HALO * 4], bf16, name="e8", tag="e8")
            # e8 groups: 4 groups of 8 j-tiles, each segment [64] wide
            ps8g = None
            for j in range(NJ):
                g, jg = divmod(j, 8)
                ps = psA.tile([P, M], f32, name=f"sps{j}", tag="ps")
                for h in range(NH):
                    nc.tensor.matmul(
                        ps,
                        kt[h][:, P * j : P * (j + 1)],
                        qt[h][:, 0:M],
                        start=(h == 0),
                        stop=(h == NH - 1),
                    )
                if jg == 0:
                    ps8g = psB.tile([P, 8 * HALO], f32, name=f"ps8g{g}", tag="ps8")
                for h in range(NH):
                    nc.tensor.matmul(
                        ps8g[:, HALO * jg : HALO * (jg + 1)],
                        kt[h][:, P * j : P * (j + 1)],
                        qt[h][:, M:MH],
                        start=(h == 0),
                        stop=(h == NH - 1),
                    )
                ej = store.tile([P, M], bf16, name=f"e{j}", tag="E", bufs=NJ)
                nc.scalar.activation(ej, ps, Exp, bias=bias_t)
                elist.append(ej)
                # interleave main Ru accumulation for this j
                for c in range(NMAIN):
                    nc.tensor.matmul(
                        ru[c],
                        elist[j][:, P * c : P * (c + 1)],
                        vc[j],
                        start=(j == 0),
                        stop=(j == NJ - 1),
                    )
                if jg == 7:
                    # tail exp for this group of 8 j-tiles, then tail Ru matmuls
                    seg = slice(8 * HALO * g, 8 * HALO * (g + 1))
                    nc.scalar.activation(e8[:, seg], ps8g, Exp, bias=bias_t)
                    for j2 in range(8 * g, 8 * (g + 1)):
                        nc.tensor.matmul(
                            ru8,
                            e8[:, HALO * j2 : HALO * (j2 + 1)],
                            vc[j2],
                            start=(j2 == 0),
                            stop=(j2 == NJ - 1),
                        )

            # ---- normalize: R = Ru[:, :320] * mask / Ru[:, 320] ----
            rch = []
            for c in range(NMAIN):
                rec = small.tile([P, 1], f32, name=f"rec{c}", tag="rec")
                if c == 0:
                    # halo rows can have rowsum 0 (zero-padded queries on core
                    # 0); add (1-mask) so 1/den is finite, then zero via mask.
                    den = small.tile([P, 1], f32, name="den0", tag="den")
                    nc.vector.tensor_add(den, ru[c][:, NS * NV : NS * NV + 1], rminv)
                    nc.vector.reciprocal(rec, den)
                    nc.vector.tensor_mul(rec, rec, rmask)
                else:
                    nc.vector.reciprocal(rec, ru[c][:, NS * NV : NS * NV + 1])
                t = store.tile([P, NS * NV], bf16, name=f"r{c}", tag=f"r{c}")
                nc.vector.tensor_scalar_mul(t, ru[c][:, 0 : NS * NV], rec)
                rch.append(t)
            rec8 = small.tile([HALO, 1], f32, name="rec8", tag="rec8")
            nc.vector.reciprocal(rec8, ru8[:, NS * NV : NS * NV + 1])
            r8 = store.tile([HALO, NS * NV], bf16, name="r8", tag="r8")
            nc.vector.tensor_scalar_mul(r8, ru8[:, 0 : NS * NV], rec8)
            rch.append(r8)

            # ---- combine: out[p, v] = sum_s w_s R[128c + p + 8 - 2s, 80s + v] ----
            for c in range(NMAIN):
                po = psA.tile([P, NV], f32, name=f"po{c}", tag="ps")
                for s in range(NS):
                    nc.tensor.matmul(
                        po,
                        sh1[:, P * s : P * (s + 1)],
                        rch[c][:, NV * s : NV * (s + 1)],
                        start=(s == 0),
                        stop=False,
                    )
                for s in range(NS):
                    nc.tensor.matmul(
                        po,
                        sh2[:, P * s : P * (s + 1)],
                        rch[c + 1][0:HALO, NV * s : NV * (s + 1)],
                        start=False,
                        stop=(s == NS - 1),
                    )
                osb = small.tile([P, NV], f32, name=f"osb{c}", tag="osb", bufs=2)
                nc.vector.tensor_copy(osb, po)
                nc.sync.dma_start(out=out_d[P * c : P * (c + 1), :], in_=osb)

    nc.compile()
    return nc


def _get_nc():
    if "nc" not in _NC_CACHE:
        _install_axon_ntff_hook()
        _NC_CACHE["nc"] = _build_nc()
    return _NC_CACHE["nc"]


def _host_prep(x, x_key, x_value, W_qk, w_shift):
    bf = ml_dtypes.bfloat16
    x = np.ascontiguousarray(np.asarray(x, dtype=np.float32))
    x_key = np.ascontiguousarray(np.asarray(x_key, dtype=np.float32))
    x_value = np.ascontiguousarray(np.asarray(x_value, dtype=np.float32))
    W_qk = np.ascontiguousarray(np.asarray(W_qk, dtype=np.float32))
    w_shift = np.asarray(w_shift, dtype=np.float32)

    xkT = np.ascontiguousarray(x_key.T)                      # [Q, T]
    wT = np.ascontiguousarray(W_qk.T)                        # [Q, H]; wT[f,h]=W[h,f]

    vcomb = np.zeros((T, NS * NV + 1), np.float32)
    for s in range(NS):
        d = STEP * s
        vcomb[: T - d, NV * s : NV * (s + 1)] = x_value[d:, :]
    vcomb[:, NS * NV] = 1.0
    vcomb = vcomb.astype(bf)

    # shmat[s][k, p] = w_s * [k == p + 8 - 2s], k in [0, 136)
    sh1 = np.zeros((P, NS, P), np.float32)
    sh2 = np.zeros((HALO, NS, P), np.float32)
    for s in range(NS):
        d = HALO - STEP * s
        for p in range(P):
            k = p + d
            if k < P:
                sh1[k, s, p] = w_shift[0, s]
            else:
                sh2[k - P, s, p] = w_shift[0, s]
    sh1 = sh1.reshape(P, NS * P).astype(bf)
    sh2 = sh2.reshape(HALO, NS * P).astype(bf)

    xpad = np.concatenate([np.zeros((HALO, Q), np.float32), x], axis=0)

    in_maps = []
    for d in range(NCORES):
        r0 = d * M
        xT = np.ascontiguousarray(xpad[r0 : r0 + MH].T)      # [Q, MH]
        rmask = np.ones((P, 1), np.float32)
        if d == 0:
            rmask[:HALO] = 0.0
        in_maps.append(
            {
                "xT": xT,
                "xkT": xkT,
                "wT": wT,
                "vcomb": vcomb,
                "shmat1": sh1,
                "shmat2": sh2,
                "rmask": rmask,
                "rminv": np.ascontiguousarray(1.0 - rmask),
            }
        )
    return in_maps


def kernel(x, x_key, x_value, W_qk, w_shift):
    global LAST_RESULT
    from concourse.bass_utils import run_bass_kernel_spmd

    nc = _get_nc()
    in_maps = _host_prep(x, x_key, x_value, W_qk, w_shift)
    res = run_bass_kernel_spmd(nc, in_maps, core_ids=list(range(NCORES)))
    LAST_RESULT = res
    out = np.concatenate([res.results[d]["out"] for d in range(NCORES)], axis=0)
    return out.astype(np.float32)


# revision 17
# speedup vs baseline: 1.2473x; 1.2473x over previous
"""Trainium2 Bass kernel for shifted sparse attention (nn_Attention_74672301408506).

Math (reference):
    q = x @ W.T ; k = x_key @ W.T ; att = softmax(q k^T)
    out[i, v] = sum_s w_s * sum_j att[i-2s, j] * x_value[j+2s, v]

Device algorithm (8 cores, query rows sharded, 8 halo rows recomputed):
    per core, with local query window rows [r0-8, r0+512):
      qT[h, i], kT[h, j]   (float32r matmuls)
      S^T[j, i] = kT^T q   (scores transposed: keys on partitions)
      E = exp(S - 110)     (bf16; softmax shift-invariant, 110 avoids overflow)
      Ru[i, 0:320|320] = E^T @ [V_0|V_1|V_2|V_3|ones]   (ones col = softmax denom)
      R = Ru[:, :320] * mask / Ru[:, 320]
      out[p, v] = sum_s w_s R[p + 8 - 2s, 80s + v]  (banded-matrix matmuls)
"""

import os
import sys
import types

import numpy as np
import ml_dtypes

T = 4096
Q = 256
H = 256
NV = 80
NS = 4
STEP = 2
NCORES = 8
M = T // NCORES            # 512 rows per core
HALO = 8                   # = (NS-1)*STEP + slack, multiple of 2
MH = M + HALO              # 520 i-window per core
CSUB = 110.0               # global score shift before exp
P = 128
NJ = T // P                # 32 key tiles
NF = Q // P                # 2 feature tiles
NH = H // P                # 2 hidden tiles
NMAIN = M // P             # 4 main i-chunks of 128


def _install_axon_ntff_hook():
    """bass_utils' trace path imports antenv.axon_hooks, which the agent image
    lacks; shim it and register the ctypes-based NTFF profiler hook."""
    if "antenv.axon_hooks" in sys.modules:
        return
    try:
        import antenv
    except ImportError:
        return
    mod = types.ModuleType("antenv.axon_hooks")
    mod._hook = None
    mod.set_axon_ntff_profile_hook = lambda h: setattr(mod, "_hook", h)
    mod.get_axon_ntff_profile_hook = lambda: mod._hook
    sys.modules["antenv.axon_hooks"] = mod
    antenv.axon_hooks = mod
    try:
        from trn_agent_boot import trn_boot

        so_path = "/opt/axon/libaxon_pjrt.so"
        if os.path.exists(so_path):
            mod.set_axon_ntff_profile_hook(trn_boot._ntff_profile_via_ctypes(so_path))
    except Exception:
        pass


_NC_CACHE = {}
LAST_RESULT = None


def _build_nc():
    import concourse.mybir as mybir
    import concourse.tile as tile
    from concourse import bacc

    f32 = mybir.dt.float32
    f32r = mybir.dt.float32r
    bf16 = mybir.dt.bfloat16
    Exp = mybir.ActivationFunctionType.Exp

    nc = bacc.Bacc(None, target_bir_lowering=False)

    xT_d = nc.dram_tensor("xT", [Q, MH], f32r, kind="ExternalInput")
    xkT_d = nc.dram_tensor("xkT", [Q, T], f32r, kind="ExternalInput")
    wT_d = nc.dram_tensor("wT", [Q, H], f32r, kind="ExternalInput")
    vc_d = nc.dram_tensor("vcomb", [T, NS * NV + 1], bf16, kind="ExternalInput")
    sh1_d = nc.dram_tensor("shmat1", [P, NS * P], bf16, kind="ExternalInput")
    sh2_d = nc.dram_tensor("shmat2", [HALO, NS * P], bf16, kind="ExternalInput")
    rmask_d = nc.dram_tensor("rmask", [P, 1], f32, kind="ExternalInput")
    rminv_d = nc.dram_tensor("rminv", [P, 1], f32, kind="ExternalInput")
    out_d = nc.dram_tensor("out", [M, NV], f32, kind="ExternalOutput")

    with tile.TileContext(nc) as tc:
        with (
            tc.tile_pool(name="consts", bufs=1) as consts,
            tc.tile_pool(name="io", bufs=3) as io,
            tc.tile_pool(name="store", bufs=1) as store,
            tc.tile_pool(name="small", bufs=6) as small,
            tc.tile_pool(name="psA", bufs=2, space="PSUM") as psA,
            tc.tile_pool(name="psB", bufs=1, space="PSUM") as psB,
            tc.tile_pool(name="psR", bufs=4, space="PSUM") as psR,
            tc.tile_pool(name="psR8", bufs=1, space="PSUM") as psR8,
        ):
            # ---- constants / small inputs (gpsimd queue; sync queue is for
            # the latency-critical xk stream) ----
            sh1 = consts.tile([P, NS * P], bf16, name="sh1")
            nc.gpsimd.dma_start(out=sh1, in_=sh1_d[:, :])
            sh2 = consts.tile([HALO, NS * P], bf16, name="sh2")
            nc.gpsimd.dma_start(out=sh2, in_=sh2_d[:, :])
            rmask = consts.tile([P, 1], f32, name="rmask")
            nc.gpsimd.dma_start(out=rmask, in_=rmask_d[:, :])
            rminv = consts.tile([P, 1], f32, name="rminv")
            nc.gpsimd.dma_start(out=rminv, in_=rminv_d[:, :])
            bias_t = consts.tile([P, 1], f32, name="bias_t")
            nc.vector.memset(bias_t, -CSUB)

            wt = []
            for f in range(NF):
                t = consts.tile([P, H], f32r, name=f"wt{f}", tag=f"wt{f}")
                nc.sync.dma_start(out=t, in_=wT_d[P * f : P * (f + 1), :])
                wt.append(t)
            xt = []
            for f in range(NF):
                t = consts.tile([P, MH], f32r, name=f"xt{f}", tag=f"xt{f}")
                nc.sync.dma_start(out=t, in_=xT_d[P * f : P * (f + 1), :])
                xt.append(t)

            # value tiles (bf16, includes ones column); gpsimd queue, in
            # parallel with the sync-queue xk stream that phase A waits on
            vc = []
            for j in range(NJ):
                t = store.tile([P, NS * NV + 1], bf16, name=f"vc{j}", tag="vc", bufs=NJ)
                nc.gpsimd.dma_start(out=t, in_=vc_d[P * j : P * (j + 1), :])
                vc.append(t)

            # ---- phase A: qT, kT ----
            qt = []
            for h in range(NH):
                ps = psA.tile([P, M], f32, name=f"qps{h}", tag="ps")
                for f in range(NF):
                    nc.tensor.matmul(
                        ps,
                        wt[f][:, P * h : P * (h + 1)],
                        xt[f][:, 0:M],
                        start=(f == 0),
                        stop=(f == NF - 1),
                    )
                ps8 = psB.tile([P, HALO], f32, name=f"qps8{h}", tag="ps8")
                for f in range(NF):
                    nc.tensor.matmul(
                        ps8,
                        wt[f][:, P * h : P * (h + 1)],
                        xt[f][:, M:MH],
                        start=(f == 0),
                        stop=(f == NF - 1),
                    )
                t = store.tile([P, MH], f32r, name=f"qt{h}", tag=f"qt{h}")
                if h == 0:
                    nc.vector.tensor_copy(t[:, 0:M], ps)
                else:
                    nc.scalar.copy(t[:, 0:M], ps)
                nc.vector.tensor_copy(t[:, M:MH], ps8)
                qt.append(t)

            kt = []
            for h in range(NH):
                kt.append(store.tile([P, T], f32r, name=f"kt{h}", tag=f"kt{h}"))
            NKC = 8  # 512-wide key chunks
            xkcs = []
            for jc in range(NKC):
                xkc = []
                for f in range(NF):
                    t = io.tile(
                        [P, 512], f32r, name=f"xkc{jc}_{f}", tag=f"xk{f}", bufs=NKC
                    )
                    nc.sync.dma_start(
                        out=t, in_=xkT_d[P * f : P * (f + 1), 512 * jc : 512 * (jc + 1)]
                    )
                    xkc.append(t)
                xkcs.append(xkc)
            for jc in range(NKC):
                xkc = xkcs[jc]
                for h in range(NH):
                    ps = psA.tile([P, 512], f32, name=f"kps{jc}_{h}", tag="ps")
                    for f in range(NF):
                        nc.tensor.matmul(
                            ps,
                            wt[f][:, P * h : P * (h + 1)],
                            xkc[f],
                            start=(f == 0),
                            stop=(f == NF - 1),
                        )
                    dst = kt[h][:, 512 * jc : 512 * (jc + 1)]
                    if (jc + h) % 2 == 0:
                        nc.vector.tensor_copy(dst, ps)
                    else:
                        nc.scalar.copy(dst, ps)

            # ---- phases B+C interleaved: scores^T -> exp -> Ru accumulation ----
            elist = []
            ru = []
            for c in range(NMAIN):
                ru.append(psR.tile([P, NS * NV + 1], f32, name=f"ru{c}", tag="ru"))
            ru8 = psR8.tile([HALO, NS * NV + 1], f32, name="ru8", tag="ru8")
            e8 = store.tile([P, 8 * HALO * 4], bf16, name="e8", tag="e8")
            # e8 groups: 4 groups of 8 j-tiles, each segment [64] wide
            ps8g = None
            for j in range(NJ):
                g, jg = divmod(j, 8)
                ps = psA.tile([P, M], f32, name=f"sps{j}", tag="ps")
                for h in range(NH):
                    nc.tensor.matmul(
                        ps,
                        kt[h][:, P * j : P * (j + 1)],
                        qt[h][:, 0:M],
                        start=(h == 0),
                        stop=(h == NH - 1),
                    )
                if jg == 0:
                    ps8g = psB.tile([P, 8 * HALO], f32, name=f"ps8g{g}", tag="ps8")
                for h in range(NH):
                    nc.tensor.matmul(
                        ps8g[:, HALO * jg : HALO * (jg + 1)],
                        kt[h][:, P * j : P * (j + 1)],
                        qt[h][:, M:MH],
                        start=(h == 0),
                        stop=(h == NH - 1),
                    )
                ej = store.tile([P, M], bf16, name=f"e{j}", tag="E", bufs=NJ)
                nc.scalar.activation(ej, ps, Exp, bias=bias_t)
                elist.append(ej)
                # interleave main Ru accumulation for this j
                for c in range(NMAIN):
                    nc.tensor.matmul(
                        ru[c],
                        elist[j][:, P * c : P * (c + 1)],
                        vc[j],
                        start=(j == 0),
                        stop=(j == NJ - 1),
                    )
                if jg == 7:
                    # tail exp for this group of 8 j-tiles, then tail Ru matmuls
                    seg = slice(8 * HALO * g, 8 * HALO * (g + 1))
                    nc.scalar.activation(e8[:, seg], ps8g, Exp, bias=bias_t)
                    for j2 in range(8 * g, 8 * (g + 1)):
                        nc.tensor.matmul(
                            ru8,
                            e8[:, HALO * j2 : HALO * (j2 + 1)],
                            vc[j2],
                            start=(j2 == 0),
                            stop=(j2 == NJ - 1),
                        )

            # ---- normalize: R = Ru[:, :320] * mask / Ru[:, 320] ----
            rch = []
            for c in range(NMAIN):
                rec = small.tile([P, 1], f32, name=f"rec{c}", tag="rec")
                if c == 0:
                    # halo rows can have rowsum 0 (zero-padded queries on core
                    # 0); add (1-mask) so 1/den is finite, then zero via mask.
                    den = small.tile([P, 1], f32, name="den0", tag="den")
                    nc.vector.tensor_add(den, ru[c][:, NS * NV : NS * NV + 1], rminv)
                    nc.vector.reciprocal(rec, den)
                    nc.vector.tensor_mul(rec, rec, rmask)
                else:
                    nc.vector.reciprocal(rec, ru[c][:, NS * NV : NS * NV + 1])
                t = store.tile([P, NS * NV], bf16, name=f"r{c}", tag=f"r{c}")
                nc.vector.tensor_scalar_mul(t, ru[c][:, 0 : NS * NV], rec)
                rch.append(t)
            rec8 = small.tile([HALO, 1], f32, name="rec8", tag="rec8")
            nc.vector.reciprocal(rec8, ru8[:, NS * NV : NS * NV + 1])
            r8 = store.tile([HALO, NS * NV], bf16, name="r8", tag="r8")
            nc.vector.tensor_scalar_mul(r8, ru8[:, 0 : NS * NV], rec8)
            rch.append(r8)

            # ---- combine: out[p, v] = sum_s w_s R[128c + p + 8 - 2s, 80s + v] ----
            for c in range(NMAIN):
                po = psA.tile([P, NV], f32, name=f"po{c}", tag="ps")
                for s in range(NS):
                    nc.tensor.matmul(
                        po,
                        sh1[:, P * s : P * (s + 1)],
                        rch[c][:, NV * s : NV * (s + 1)],
                        start=(s == 0),
                        stop=False,
                    )
                for s in range(NS):
                    nc.tensor.matmul(
                        po,
                        sh2[:, P * s : P * (s + 1)],
                        rch[c + 1][0:HALO, NV * s : NV * (s + 1)],
                        start=False,
                        stop=(s == NS - 1),
                    )
                osb = small.tile([P, NV], f32, name=f"osb{c}", tag="osb", bufs=2)
                nc.vector.tensor_copy(osb, po)
                nc.sync.dma_start(out=out_d[P * c : P * (c + 1), :], in_=osb)

    nc.compile()
    return nc


def _get_nc():
    if "nc" not in _NC_CACHE:
        _install_axon_ntff_hook()
        _NC_CACHE["nc"] = _build_nc()
    return _NC_CACHE["nc"]


def _host_prep(x, x_key, x_value, W_qk, w_shift):
    bf = ml_dtypes.bfloat16
    x = np.ascontiguousarray(np.asarray(x, dtype=np.float32))
    x_key = np.ascontiguousarray(np.asarray(x_key, dtype=np.float32))
    x_value = np.ascontiguousarray(np.asarray(x_value, dtype=np.float32))
    W_qk = np.ascontiguousarray(np.asarray(W_qk, dtype=np.float32))
    w_shift = np.asarray(w_shift, dtype=np.float32)

    xkT = np.ascontiguousarray(x_key.T)                      # [Q, T]
    wT = np.ascontiguousarray(W_qk.T)                        # [Q, H]; wT[f,h]=W[h,f]

    vcomb = np.zeros((T, NS * NV + 1), np.float32)
    for s in range(NS):
        d = STEP * s
        vcomb[: T - d, NV * s : NV * (s + 1)] = x_value[d:, :]
    vcomb[:, NS * NV] = 1.0
    vcomb = vcomb.astype(bf)

    # shmat[s][k, p] = w_s * [k == p + 8 - 2s], k in [0, 136)
    sh1 = np.zeros((P, NS, P), np.float32)
    sh2 = np.zeros((HALO, NS, P), np.float32)
    for s in range(NS):
        d = HALO - STEP * s
        for p in range(P):
            k = p + d
            if k < P:
                sh1[k, s, p] = w_shift[0, s]
            else:
                sh2[k - P, s, p] = w_shift[0, s]
    sh1 = sh1.reshape(P, NS * P).astype(bf)
    sh2 = sh2.reshape(HALO, NS * P).astype(bf)

    xpad = np.concatenate([np.zeros((HALO, Q), np.float32), x], axis=0)

    in_maps = []
    for d in range(NCORES):
        r0 = d * M
        xT = np.ascontiguousarray(xpad[r0 : r0 + MH].T)      # [Q, MH]
        rmask = np.ones((P, 1), np.float32)
        if d == 0:
            rmask[:HALO] = 0.0
        in_maps.append(
            {
                "xT": xT,
                "xkT": xkT,
                "wT": wT,
                "vcomb": vcomb,
                "shmat1": sh1,
                "shmat2": sh2,
                "rmask": rmask,
                "rminv": np.ascontiguousarray(1.0 - rmask),
            }
        )
    return in_maps


def kernel(x, x_key, x_value, W_qk, w_shift):
    global LAST_RESULT
    from concourse.bass_utils import run_bass_kernel_spmd

    nc = _get_nc()
    in_maps = _host_prep(x, x_key, x_value, W_qk, w_shift)
    res = run_bass_kernel_spmd(nc, in_maps, core_ids=list(range(NCORES)))
    LAST_RESULT = res
    out = np.concatenate([res.results[d]["out"] for d in range(NCORES)], axis=0)
    return out.astype(np.float32)
